# revision 27
# baseline (speedup 1.0000x reference)
"""Trainium2 Bass kernel: separable parabolic morphological dilation (11-tap).

nn_Dilation2dSingle: im [8, 32, 512, 512] f32, se_coef scalar, se [11, 1].
    bias[k] = se_coef * se[k, 0]           (parabolic, symmetric, bias[5] = 0)
    out = vdilate(hdilate(im)) with NEG=-10000 padding.

Strategy (v2, fp16):
  * Host converts im to fp16 and pre-transposes each sample to [C, W, H], so
    the kernel runs the vertical pass first (along the free axis), does ONE
    on-device transpose (PE identity-matmul into PSUM, ACT copy back), runs
    the horizontal pass, and stores [C, H, W] directly -- no second transpose.
  * Each 1D pass is computed in unfused form to exploit DVE perf modes:
      - one batched tensor_max over a strided 4D AP computes all 5 symmetric
        pair maxima p_d = max(x[i-d], x[i+d]) in a single 2x-mode instruction
      - the 5 bias subtracts q_d = p_d - b_d are single-src ops: 4x mode on
        DVE (tensor_scalar) or offloaded to ACT (Identity activation with a
        per-partition bias AP)
      - a 4-instruction max tree combines q_1..q_5 and the center tap
    This is ~6.25 DVE-cyc/elem/pass vs 15 for the naive fused form
    (scalar_tensor_tensor has no 2x uop; tensor_max does).
  * GPSIMD cannot help: on TRN2 walrus only accepts float add/sub/mult (no
    max) TensorTensor on the Pool engine, and InstPool must run on DVE; the
    relu-max decomposition (GP sub + ACT relu + GP add) loses to its
    cross-engine latency. So DVE carries all max work and ACT the subtracts.
  * Bias values are compiled in as immediates/constants derived from the
    runtime se_coef (the module cache is keyed on them).

Sharding: pure data-parallel over batch (8 cores x 1 batch each).
"""

from contextlib import ExitStack

import numpy as np

import concourse.bacc as bacc
import concourse.mybir as mybir
import concourse.tile as tile
from concourse.bass_utils import run_bass_kernel_spmd

F16 = mybir.dt.float16
F32 = mybir.dt.float32
NEG = -10000.0
R = 5  # dilation radius (window 11)

# Hardcoded problem shape (per spec).
B, C, H, W = 8, 32, 512, 512
N_CORES = 8
CP = 4  # channels packed per instruction group
L = 512  # pass length (H == W == 512)
SL = L + 2 * R
nT = L // 128  # partition tiles per pass
nG = C // CP  # channel groups

def _gp_tile(g, k):
    """Tile mode for tile k (0-3 pass 1, 4-7 pass 2) of group g. "dve" is
    the only profitable mode on TRN2 (see header); "assist"/"assist_dsub"
    (relu-max via GPSIMD+ACT) are kept for experimentation but lose to
    cross-engine latency."""
    return "dve"
ACT_SUBS = True  # bias subtracts on ACT (True) or DVE 4x tensor_scalar
PADS_ONCE = True  # write NEG halos once per pool buffer instead of per tile


def _win_pair_aps(src3, pd_view, nd=R):
    """Build the (left, right) strided APs covering the first nd shifted taps.

    src3: AP [128, CP, SL]; returns 4D APs [128, nd, CP, L] where dim 1 walks
    d=1..nd via stride -1/+1 starting at offset R-1 / R+1.
    """
    left = src3[:, :, R - 1 : R - 1 + L].unsqueeze(1).copy()
    left.ap[1] = [-1, nd]
    right = src3[:, :, R + 1 : R + 1 + L].unsqueeze(1).copy()
    right.ap[1] = [1, nd]
    return left, right


def _dilate_dve(nc, pd_pool, t_pool, out_v, src3, biasneg_t, bias_vals, use_act,
                assist=False, deferred=None):
    """Unfused pair/sub/tree dilation on DVE (+ACT for the subtracts).

    With assist=True the d=4,5 pair maxima are computed as
    x[i+d] + relu(x[i-d] - x[i+d]): the subtract and add run on GPSIMD
    (the only float elementwise ops walrus accepts on Pool) and the relu on
    ACT, freeing DVE cycles. The GP adds are deferred to the next dilate
    call so GPSIMD always has the following tile's diffs queued while an
    add waits on its relu."""
    pd = pd_pool.tile([128, R * CP * L], F16, tag="pd")
    pd_v = pd[:].rearrange("p (d s c) -> p d s c", d=R, s=CP)
    nd = R - 2 if assist else R
    if assist:
        for d in (4, 5):
            slot = pd_v[:, d - 1]
            nc.gpsimd.tensor_tensor(
                slot,
                src3[:, :, R - d : R - d + L],
                src3[:, :, R + d : R + d + L],
                op=mybir.AluOpType.subtract,
            )
    left, right = _win_pair_aps(src3, pd_v, nd)
    nc.vector.tensor_tensor(
        pd_v[:, 0:nd], left, right, op=mybir.AluOpType.max
    )
    if assist:
        for d in (4, 5):
            slot = pd_v[:, d - 1]
            nc.scalar.activation(slot, slot, mybir.ActivationFunctionType.Relu)
        for d in (4, 5):
            slot = pd_v[:, d - 1]
            nc.gpsimd.tensor_tensor(
                slot, slot,
                src3[:, :, R + d : R + d + L],
                op=mybir.AluOpType.add,
            )
    for d in range(1, nd + 1):
        q = pd_v[:, d - 1]
        if use_act:
            nc.scalar.activation(
                q, q, mybir.ActivationFunctionType.Identity,
                bias=biasneg_t[:, d - 1 : d],
            )
        else:
            nc.vector.tensor_scalar_add(q, q, -bias_vals[d - 1])
    if assist:
        for d in (4, 5):
            nc.vector.tensor_scalar_add(
                pd_v[:, d - 1], pd_v[:, d - 1], -bias_vals[d - 1]
            )
    # max tree over {q1..q5, center}
    t12 = t_pool.tile([128, 2 * CP * L], F16, tag="t12")
    t12_v = t12[:].rearrange("p (d s c) -> p d s c", d=2, s=CP)
    qa = pd_v[:, 0:1].copy()
    qa.ap[1] = [2 * CP * L, 2]  # q1, q3
    qb = pd_v[:, 1:2].copy()
    qb.ap[1] = [2 * CP * L, 2]  # q2, q4
    nc.vector.tensor_tensor(t12_v, qa, qb, op=mybir.AluOpType.max)
    u1 = pd_v[:, 0]
    nc.vector.tensor_tensor(u1, t12_v[:, 0], t12_v[:, 1], op=mybir.AluOpType.max)
    u2 = pd_v[:, 1]
    center = src3[:, :, R : R + L]
    nc.vector.tensor_tensor(u2, pd_v[:, R - 1], center, op=mybir.AluOpType.max)
    nc.vector.tensor_tensor(out_v, u1, u2, op=mybir.AluOpType.max)


def build_nc(bias_vals, reps=1):
    """Build the per-core Bass module. bias_vals: tuple of 5 floats (b_1..b_5).

    reps > 1 repeats the whole pipeline (same output) -- used only for
    differential timing experiments."""
    nc = bacc.Bacc("TRN2", target_bir_lowering=False, debug=False)
    imt = nc.dram_tensor("imt", [C, W, H], F16, kind="ExternalInput")
    biasneg = nc.dram_tensor("biasneg", [128, R], F32, kind="ExternalInput")
    iden = nc.dram_tensor("iden", [128, 128], F16, kind="ExternalInput")
    out = nc.dram_tensor("out", [C, H, W], F16, kind="ExternalOutput")

    with tile.TileContext(nc) as tc, ExitStack() as ctx:
        const_pool = ctx.enter_context(tc.tile_pool(name="const", bufs=1))
        hin_pool = ctx.enter_context(tc.tile_pool(name="hin", bufs=6))
        vin_pool = ctx.enter_context(tc.tile_pool(name="vin", bufs=5))
        pd_pool = ctx.enter_context(tc.tile_pool(name="pd", bufs=3))
        t_pool = ctx.enter_context(tc.tile_pool(name="t12", bufs=2))
        # pass 2 of group g is emitted after pass 1 of g+1 (software
        # pipelining), so two groups' worth of pass-1 results are live.
        hacc_pool = ctx.enter_context(tc.tile_pool(name="hacc", bufs=2 * nT + 1))
        st_pool = ctx.enter_context(tc.tile_pool(name="st", bufs=4))
        psf_pool = ctx.enter_context(tc.tile_pool(name="psf", bufs=3, space="PSUM"))

        identity = const_pool.tile([128, 128], F16)
        nc.sync.dma_start(identity[:], iden.ap())
        biasneg_t = const_pool.tile([128, R], F32)
        nc.sync.dma_start(biasneg_t[:], biasneg.ap())
        neg_t = const_pool.tile([128, CP * R], F16)
        nc.gpsimd.memset(neg_t[:], NEG)

        pad_seen = {}

        def set_pads(tile_, tag):
            """Write NEG into the halo pads; once per physical buffer if
            PADS_ONCE (pool buffers rotate round-robin, pads are never
            overwritten afterwards)."""
            n = pad_seen.get(tag, 0)
            if PADS_ONCE and n >= 6:
                return
            pad_seen[tag] = n + 1
            v = tile_[:].rearrange("p (s c) -> p s c", s=CP)
            src = neg_t[:].rearrange("p (s c) -> p s c", s=CP)
            nc.scalar.copy(v[:, :, 0:R], src)
            nc.scalar.copy(v[:, :, SL - R : SL], src)

        def dilate(eng, out_v, src3):
            _dilate_dve(
                nc, pd_pool, t_pool, out_v, src3, biasneg_t, bias_vals,
                use_act=(eng != "assist_dsub"),
                assist=eng.startswith("assist"),
            )

        def pass1(g):
            haccs = []
            for t in range(nT):
                ht = hin_pool.tile([128, CP * SL], F16, tag="hin")
                set_pads(ht, "hin")
                src = imt.ap()[
                    g * CP : (g + 1) * CP, t * 128 : (t + 1) * 128, :
                ].rearrange("c w h -> w c h")
                hv = ht[:].rearrange("p (s c) -> p s c", s=CP)
                nc.sync.dma_start(hv[:, :, R : R + L], src)
                acc = hacc_pool.tile([128, CP * L], F16, tag="hacc")
                accv = acc[:].rearrange("p (s c) -> p s c", s=CP)
                dilate(_gp_tile(g, t), accv, hv)
                haccs.append(acc)
            return haccs

        def pass2(g, haccs):
            for t2 in range(nT):
                vt = vin_pool.tile([128, CP * SL], F16, tag="vin")
                set_pads(vt, "vin")
                vv = vt[:].rearrange("p (s c) -> p s c", s=CP)
                pt = psf_pool.tile([128, CP * L], F16, tag="psf")
                pt_v = pt[:].rearrange("p (s c) -> p s c", s=CP)
                for ci in range(CP):
                    for t1 in range(nT):
                        nc.tensor.transpose(
                            pt_v[:, ci, t1 * 128 : (t1 + 1) * 128],
                            haccs[t1][:, ci * L + t2 * 128 : ci * L + (t2 + 1) * 128],
                            identity[:],
                        )
                nc.scalar.copy(vv[:, :, R : R + L], pt_v)
                stt = st_pool.tile([128, CP * L], F16, tag="st")
                stv = stt[:].rearrange("p (s c) -> p s c", s=CP)
                dilate(_gp_tile(g, nT + t2), stv, vv)
                dst = out.ap()[
                    g * CP : (g + 1) * CP, t2 * 128 : (t2 + 1) * 128, :
                ].rearrange("c h w -> h c w")
                nc.sync.dma_start(dst, stv)

        for _rep in range(reps):
            # Software pipeline: pass 2 of group g is emitted after pass 1 of
            # group g+1, so every engine always has two groups of independent
            # work in its instruction window.
            prev = None
            for g in range(nG):
                haccs = pass1(g)
                if prev is not None:
                    pass2(prev[0], prev[1])
                prev = (g, haccs)
            pass2(prev[0], prev[1])

    nc.compile()
    return nc


_NC_CACHE = {}


def _get_nc(bias_vals=None):
    if bias_vals is None:
        bias_vals = next(iter(_NC_CACHE))
    if bias_vals not in _NC_CACHE:
        _NC_CACHE[bias_vals] = build_nc(bias_vals)
    return _NC_CACHE[bias_vals]


def _bias_vals(se_coef, se):
    se = np.asarray(se, dtype=np.float32)
    se_coef = np.asarray(se_coef, dtype=np.float32)
    bias11 = (se_coef * se[:, 0]).astype(np.float32)  # same fp32 op as reference
    return tuple(float(x) for x in bias11[R + 1 : 2 * R + 1])


def _make_in_maps(im, se_coef, se):
    im = np.asarray(im, dtype=np.float32)
    bias_vals = _bias_vals(se_coef, se)
    biasneg = np.ascontiguousarray(
        np.broadcast_to(-np.asarray(bias_vals, dtype=np.float32), (128, R))
    )
    iden = np.eye(128, dtype=np.float16)
    imt = np.ascontiguousarray(np.swapaxes(im, 2, 3)).astype(np.float16)
    return [
        {"imt": imt[b], "biasneg": biasneg, "iden": iden} for b in range(im.shape[0])
    ]


def kernel(im, se_coef, se):
    bias_vals = _bias_vals(se_coef, se)
    nc = _get_nc(bias_vals)
    in_maps = _make_in_maps(im, se_coef, se)
    res = run_bass_kernel_spmd(nc, in_maps, core_ids=list(range(N_CORES)))
    out = np.stack([res.results[b]["out"] for b in range(N_CORES)], axis=0)
    return out.astype(np.float32)


# revision 30
# speedup vs baseline: 1.0015x; 1.0015x over previous
"""Trainium2 Bass kernel: separable parabolic morphological dilation (11-tap).

nn_Dilation2dSingle: im [8, 32, 512, 512] f32, se_coef scalar, se [11, 1].
    bias[k] = se_coef * se[k, 0]           (parabolic, symmetric, bias[5] = 0)
    out = vdilate(hdilate(im)) with NEG=-10000 padding.

Strategy (v2, fp16):
  * Host converts im to fp16 and pre-transposes each sample to [C, W, H], so
    the kernel runs the vertical pass first (along the free axis), does ONE
    on-device transpose (PE identity-matmul into PSUM, ACT copy back), runs
    the horizontal pass, and stores [C, H, W] directly -- no second transpose.
  * Each 1D pass is computed in unfused form to exploit DVE perf modes:
      - one batched tensor_max over a strided 4D AP computes all 5 symmetric
        pair maxima p_d = max(x[i-d], x[i+d]) in a single 2x-mode instruction
      - the 5 bias subtracts q_d = p_d - b_d are single-src ops: 4x mode on
        DVE (tensor_scalar) or offloaded to ACT (Identity activation with a
        per-partition bias AP)
      - a 4-instruction max tree combines q_1..q_5 and the center tap
    This is ~6.25 DVE-cyc/elem/pass vs 15 for the naive fused form
    (scalar_tensor_tensor has no 2x uop; tensor_max does).
  * GPSIMD cannot help: on TRN2 walrus only accepts float add/sub/mult (no
    max) TensorTensor on the Pool engine, and InstPool must run on DVE; the
    relu-max decomposition (GP sub + ACT relu + GP add) loses to its
    cross-engine latency. So DVE carries all max work and ACT the subtracts.
  * Bias values are compiled in as immediates/constants derived from the
    runtime se_coef (the module cache is keyed on them).

Sharding: pure data-parallel over batch (8 cores x 1 batch each).
"""

from contextlib import ExitStack

import numpy as np

import concourse.bacc as bacc
import concourse.mybir as mybir
import concourse.tile as tile
from concourse.bass_utils import run_bass_kernel_spmd

F16 = mybir.dt.float16
F32 = mybir.dt.float32
NEG = -10000.0
R = 5  # dilation radius (window 11)

# Hardcoded problem shape (per spec).
B, C, H, W = 8, 32, 512, 512
N_CORES = 8
CP = 4  # channels packed per instruction group
L = 512  # pass length (H == W == 512)
SL = L + 2 * R
nT = L // 128  # partition tiles per pass
nG = C // CP  # channel groups

def _gp_tile(g, k):
    """Tile mode for tile k (0-3 pass 1, 4-7 pass 2) of group g. "dve" is
    the only profitable mode on TRN2 (see header); "assist"/"assist_dsub"
    (relu-max via GPSIMD+ACT) are kept for experimentation but lose to
    cross-engine latency."""
    return "dve"
ACT_SUBS = True  # bias subtracts on ACT (True) or DVE 4x tensor_scalar
PADS_ONCE = True  # write NEG halos once per pool buffer instead of per tile


def _win_pair_aps(src3, pd_view, nd=R):
    """Build the (left, right) strided APs covering the first nd shifted taps.

    src3: AP [128, CP, SL]; returns 4D APs [128, nd, CP, L] where dim 1 walks
    d=1..nd via stride -1/+1 starting at offset R-1 / R+1.
    """
    left = src3[:, :, R - 1 : R - 1 + L].unsqueeze(1).copy()
    left.ap[1] = [-1, nd]
    right = src3[:, :, R + 1 : R + 1 + L].unsqueeze(1).copy()
    right.ap[1] = [1, nd]
    return left, right


def _dilate_dve(nc, pd_v, out_v, src3, biasneg_t, bias_vals):
    """Unfused pair/sub/tree dilation on DVE (+ACT for the subtracts).

    pd_v: [128, 6, CP, L] scratch whose slot 5 already holds the center tap
    (written by an off-engine SBUF->SBUF DMA at input-ready time). The
    6-leaf max tree then runs as a 3-instruction in-place block cascade:
    slots (0,1),(2,3),(4,5) -> slots 0,1,2; (0,1) -> 0; (0,2) -> out.
    In-place is safe: each block's reads finish a full CP*L-element stride
    before any later block overwrites that region."""
    left, right = _win_pair_aps(src3, pd_v)
    nc.vector.tensor_tensor(
        pd_v[:, 0:R], left, right, op=mybir.AluOpType.max
    )
    for d in range(1, R + 1):
        q = pd_v[:, d - 1]
        nc.scalar.activation(
            q, q, mybir.ActivationFunctionType.Identity,
            bias=biasneg_t[:, d - 1 : d],
        )
    sa = pd_v[:, 0:1].copy()
    sa.ap[1] = [2 * CP * L, 3]  # slots 0, 2, 4
    sb = pd_v[:, 1:2].copy()
    sb.ap[1] = [2 * CP * L, 3]  # slots 1, 3, 5
    nc.vector.tensor_tensor(pd_v[:, 0:3], sa, sb, op=mybir.AluOpType.max)
    nc.vector.tensor_tensor(pd_v[:, 0], pd_v[:, 0], pd_v[:, 1],
                            op=mybir.AluOpType.max)
    nc.vector.tensor_tensor(out_v, pd_v[:, 0], pd_v[:, 2],
                            op=mybir.AluOpType.max)


def build_nc(bias_vals, reps=1):
    """Build the per-core Bass module. bias_vals: tuple of 5 floats (b_1..b_5).

    reps > 1 repeats the whole pipeline (same output) -- used only for
    differential timing experiments."""
    nc = bacc.Bacc("TRN2", target_bir_lowering=False, debug=False)
    imt = nc.dram_tensor("imt", [C, W, H], F16, kind="ExternalInput")
    biasneg = nc.dram_tensor("biasneg", [128, R], F32, kind="ExternalInput")
    iden = nc.dram_tensor("iden", [128, 128], F16, kind="ExternalInput")
    out = nc.dram_tensor("out", [C, H, W], F16, kind="ExternalOutput")

    with tile.TileContext(nc) as tc, ExitStack() as ctx:
        const_pool = ctx.enter_context(tc.tile_pool(name="const", bufs=1))
        hin_pool = ctx.enter_context(tc.tile_pool(name="hin", bufs=6))
        vin_pool = ctx.enter_context(tc.tile_pool(name="vin", bufs=5))
        pd_pool = ctx.enter_context(tc.tile_pool(name="pd", bufs=4))
        # pass 2 of group g is emitted after pass 1 of g+1 (software
        # pipelining), so two groups' worth of pass-1 results are live.
        hacc_pool = ctx.enter_context(tc.tile_pool(name="hacc", bufs=2 * nT + 1))
        st_pool = ctx.enter_context(tc.tile_pool(name="st", bufs=4))
        psf_pool = ctx.enter_context(tc.tile_pool(name="psf", bufs=3, space="PSUM"))

        identity = const_pool.tile([128, 128], F16)
        nc.sync.dma_start(identity[:], iden.ap())
        biasneg_t = const_pool.tile([128, R], F32)
        nc.sync.dma_start(biasneg_t[:], biasneg.ap())
        neg_t = const_pool.tile([128, CP * R], F16)
        nc.gpsimd.memset(neg_t[:], NEG)

        pad_seen = {}

        def set_pads(tile_, tag):
            """Write NEG into the halo pads; once per physical buffer if
            PADS_ONCE (pool buffers rotate round-robin, pads are never
            overwritten afterwards)."""
            n = pad_seen.get(tag, 0)
            if PADS_ONCE and n >= 6:
                return
            pad_seen[tag] = n + 1
            v = tile_[:].rearrange("p (s c) -> p s c", s=CP)
            src = neg_t[:].rearrange("p (s c) -> p s c", s=CP)
            nc.scalar.copy(v[:, :, 0:R], src)
            nc.scalar.copy(v[:, :, SL - R : SL], src)

        def alloc_pd(src3, dram_src=None):
            """Allocate the 6-slot scratch and DMA the center tap into slot 5
            while other engines are busy. Pass 1 sources it straight from
            DRAM (independent of the halo load); pass 2 copies SBUF->SBUF
            from the transposed tile."""
            pd = pd_pool.tile([128, 6 * CP * L], F16, tag="pd")
            pd_v = pd[:].rearrange("p (d s c) -> p d s c", d=6, s=CP)
            nc.sync.dma_start(
                pd_v[:, R],
                dram_src if dram_src is not None else src3[:, :, R : R + L],
            )
            return pd_v

        def dilate(eng, out_v, src3, pd_v):
            _dilate_dve(nc, pd_v, out_v, src3, biasneg_t, bias_vals)

        def pass1(g):
            haccs = []
            for t in range(nT):
                ht = hin_pool.tile([128, CP * SL], F16, tag="hin")
                set_pads(ht, "hin")
                src = imt.ap()[
                    g * CP : (g + 1) * CP, t * 128 : (t + 1) * 128, :
                ].rearrange("c w h -> w c h")
                hv = ht[:].rearrange("p (s c) -> p s c", s=CP)
                nc.sync.dma_start(hv[:, :, R : R + L], src)
                pd_v = alloc_pd(hv, dram_src=src)
                acc = hacc_pool.tile([128, CP * L], F16, tag="hacc")
                accv = acc[:].rearrange("p (s c) -> p s c", s=CP)
                dilate(_gp_tile(g, t), accv, hv, pd_v)
                haccs.append(acc)
            return haccs

        def pass2(g, haccs):
            for t2 in range(nT):
                vt = vin_pool.tile([128, CP * SL], F16, tag="vin")
                set_pads(vt, "vin")
                vv = vt[:].rearrange("p (s c) -> p s c", s=CP)
                pt = psf_pool.tile([128, CP * L], F16, tag="psf")
                pt_v = pt[:].rearrange("p (s c) -> p s c", s=CP)
                for ci in range(CP):
                    for t1 in range(nT):
                        nc.tensor.transpose(
                            pt_v[:, ci, t1 * 128 : (t1 + 1) * 128],
                            haccs[t1][:, ci * L + t2 * 128 : ci * L + (t2 + 1) * 128],
                            identity[:],
                        )
                nc.scalar.copy(vv[:, :, R : R + L], pt_v)
                pd_v = alloc_pd(vv)
                stt = st_pool.tile([128, CP * L], F16, tag="st")
                stv = stt[:].rearrange("p (s c) -> p s c", s=CP)
                dilate(_gp_tile(g, nT + t2), stv, vv, pd_v)
                dst = out.ap()[
                    g * CP : (g + 1) * CP, t2 * 128 : (t2 + 1) * 128, :
                ].rearrange("c h w -> h c w")
                nc.sync.dma_start(dst, stv)

        for _rep in range(reps):
            # Software pipeline: pass 2 of group g is emitted after pass 1 of
            # group g+1, so every engine always has two groups of independent
            # work in its instruction window.
            prev = None
            for g in range(nG):
                haccs = pass1(g)
                if prev is not None:
                    pass2(prev[0], prev[1])
                prev = (g, haccs)
            pass2(prev[0], prev[1])

    nc.compile()
    return nc


_NC_CACHE = {}


def _get_nc(bias_vals=None):
    if bias_vals is None:
        bias_vals = next(iter(_NC_CACHE))
    if bias_vals not in _NC_CACHE:
        _NC_CACHE[bias_vals] = build_nc(bias_vals)
    return _NC_CACHE[bias_vals]


def _bias_vals(se_coef, se):
    se = np.asarray(se, dtype=np.float32)
    se_coef = np.asarray(se_coef, dtype=np.float32)
    bias11 = (se_coef * se[:, 0]).astype(np.float32)  # same fp32 op as reference
    return tuple(float(x) for x in bias11[R + 1 : 2 * R + 1])


def _make_in_maps(im, se_coef, se):
    im = np.asarray(im, dtype=np.float32)
    bias_vals = _bias_vals(se_coef, se)
    biasneg = np.ascontiguousarray(
        np.broadcast_to(-np.asarray(bias_vals, dtype=np.float32), (128, R))
    )
    iden = np.eye(128, dtype=np.float16)
    imt = np.ascontiguousarray(np.swapaxes(im, 2, 3)).astype(np.float16)
    return [
        {"imt": imt[b], "biasneg": biasneg, "iden": iden} for b in range(im.shape[0])
    ]


def kernel(im, se_coef, se):
    bias_vals = _bias_vals(se_coef, se)
    nc = _get_nc(bias_vals)
    in_maps = _make_in_maps(im, se_coef, se)
    res = run_bass_kernel_spmd(nc, in_maps, core_ids=list(range(N_CORES)))
    out = np.stack([res.results[b]["out"] for b in range(N_CORES)], axis=0)
    return out.astype(np.float32)


# revision 31
# speedup vs baseline: 1.0023x; 1.0007x over previous
"""Trainium2 Bass kernel: separable parabolic morphological dilation (11-tap).

nn_Dilation2dSingle: im [8, 32, 512, 512] f32, se_coef scalar, se [11, 1].
    bias[k] = se_coef * se[k, 0]           (parabolic, symmetric, bias[5] = 0)
    out = vdilate(hdilate(im)) with NEG=-10000 padding.

Strategy (v2, fp16):
  * Host converts im to fp16 and pre-transposes each sample to [C, W, H], so
    the kernel runs the vertical pass first (along the free axis), does ONE
    on-device transpose (PE identity-matmul into PSUM, ACT copy back), runs
    the horizontal pass, and stores [C, H, W] directly -- no second transpose.
  * Each 1D pass is computed in unfused form to exploit DVE perf modes:
      - one batched tensor_max over a strided 4D AP computes all 5 symmetric
        pair maxima p_d = max(x[i-d], x[i+d]) in a single 2x-mode instruction
      - the 5 bias subtracts q_d = p_d - b_d are single-src ops: 4x mode on
        DVE (tensor_scalar) or offloaded to ACT (Identity activation with a
        per-partition bias AP)
      - a 4-instruction max tree combines q_1..q_5 and the center tap
    This is ~6.25 DVE-cyc/elem/pass vs 15 for the naive fused form
    (scalar_tensor_tensor has no 2x uop; tensor_max does).
  * GPSIMD cannot help: on TRN2 walrus only accepts float add/sub/mult (no
    max) TensorTensor on the Pool engine, and InstPool must run on DVE; the
    relu-max decomposition (GP sub + ACT relu + GP add) loses to its
    cross-engine latency. So DVE carries all max work and ACT the subtracts.
  * Bias values are compiled in as immediates/constants derived from the
    runtime se_coef (the module cache is keyed on them).

Sharding: pure data-parallel over batch (8 cores x 1 batch each).
"""

from contextlib import ExitStack

import numpy as np

import concourse.bacc as bacc
import concourse.mybir as mybir
import concourse.tile as tile
from concourse.bass_utils import run_bass_kernel_spmd

F16 = mybir.dt.float16
F32 = mybir.dt.float32
NEG = -10000.0
R = 5  # dilation radius (window 11)

# Hardcoded problem shape (per spec).
B, C, H, W = 8, 32, 512, 512
N_CORES = 8
CP = 4  # channels packed per instruction group
L = 512  # pass length (H == W == 512)
SL = L + 2 * R
nT = L // 128  # partition tiles per pass
nG = C // CP  # channel groups

def _gp_tile(g, k):
    """Tile mode for tile k (0-3 pass 1, 4-7 pass 2) of group g. "dve" is
    the only profitable mode on TRN2 (see header); "assist"/"assist_dsub"
    (relu-max via GPSIMD+ACT) are kept for experimentation but lose to
    cross-engine latency."""
    return "dve"
ACT_SUBS = True  # bias subtracts on ACT (True) or DVE 4x tensor_scalar
PADS_ONCE = True  # write NEG halos once per pool buffer instead of per tile


def _win_pair_aps(src3, pd_view, nd=R):
    """Build the (left, right) strided APs covering the first nd shifted taps.

    src3: AP [128, CP, SL]; returns 4D APs [128, nd, CP, L] where dim 1 walks
    d=1..nd via stride -1/+1 starting at offset R-1 / R+1.
    """
    left = src3[:, :, R - 1 : R - 1 + L].unsqueeze(1).copy()
    left.ap[1] = [-1, nd]
    right = src3[:, :, R + 1 : R + 1 + L].unsqueeze(1).copy()
    right.ap[1] = [1, nd]
    return left, right


def _dilate_dve(nc, pd_v, out_v, src3, biasneg_t, bias_vals,
                center_src=None, split_out=None):
    """Unfused pair/sub/tree dilation on DVE (+ACT for the subtracts).

    pd_v: [128, 6, CP, L] scratch whose slot 5 already holds the center tap
    (written by an off-engine SBUF->SBUF DMA at input-ready time). The
    6-leaf max tree then runs as a 3-instruction in-place block cascade:
    slots (0,1),(2,3),(4,5) -> slots 0,1,2; (0,1) -> 0; (0,2) -> out.
    In-place is safe: each block's reads finish a full CP*L-element stride
    before any later block overwrites that region."""
    left, right = _win_pair_aps(src3, pd_v)
    nc.vector.tensor_tensor(
        pd_v[:, 0:R], left, right, op=mybir.AluOpType.max
    )
    for d in range(1, R + 1):
        q = pd_v[:, d - 1]
        nc.scalar.activation(
            q, q, mybir.ActivationFunctionType.Identity,
            bias=biasneg_t[:, d - 1 : d],
        )
    if center_src is not None:
        # first-tile variant: no center DMA (it would serialize behind the
        # very first load); classic 4-instruction tree reading the center
        # from the input tile instead.
        sa = pd_v[:, 0:1].copy()
        sa.ap[1] = [2 * CP * L, 2]  # slots 0, 2
        sb = pd_v[:, 1:2].copy()
        sb.ap[1] = [2 * CP * L, 2]  # slots 1, 3
        nc.vector.tensor_tensor(pd_v[:, 0:2], sa, sb, op=mybir.AluOpType.max)
        nc.vector.tensor_tensor(pd_v[:, 0], pd_v[:, 0], pd_v[:, 1],
                                op=mybir.AluOpType.max)
        nc.vector.tensor_tensor(pd_v[:, 1], pd_v[:, 4], center_src,
                                op=mybir.AluOpType.max)
        nc.vector.tensor_tensor(out_v, pd_v[:, 0], pd_v[:, 1],
                                op=mybir.AluOpType.max)
        return
    sa = pd_v[:, 0:1].copy()
    sa.ap[1] = [2 * CP * L, 3]  # slots 0, 2, 4
    sb = pd_v[:, 1:2].copy()
    sb.ap[1] = [2 * CP * L, 3]  # slots 1, 3, 5
    nc.vector.tensor_tensor(pd_v[:, 0:3], sa, sb, op=mybir.AluOpType.max)
    nc.vector.tensor_tensor(pd_v[:, 0], pd_v[:, 0], pd_v[:, 1],
                            op=mybir.AluOpType.max)
    if split_out is None:
        nc.vector.tensor_tensor(out_v, pd_v[:, 0], pd_v[:, 2],
                                op=mybir.AluOpType.max)
    else:
        # final-tile variant: emit the last max in two channel halves with
        # the store callback between them, so half the store overlaps the
        # second half of the compute (shrinks the pipeline tail).
        h = CP // 2
        nc.vector.tensor_tensor(out_v[:, 0:h], pd_v[:, 0, 0:h],
                                pd_v[:, 2, 0:h], op=mybir.AluOpType.max)
        split_out(0, h)
        nc.vector.tensor_tensor(out_v[:, h:CP], pd_v[:, 0, h:CP],
                                pd_v[:, 2, h:CP], op=mybir.AluOpType.max)
        split_out(h, CP)


def build_nc(bias_vals, reps=1):
    """Build the per-core Bass module. bias_vals: tuple of 5 floats (b_1..b_5).

    reps > 1 repeats the whole pipeline (same output) -- used only for
    differential timing experiments."""
    nc = bacc.Bacc("TRN2", target_bir_lowering=False, debug=False)
    imt = nc.dram_tensor("imt", [C, W, H], F16, kind="ExternalInput")
    biasneg = nc.dram_tensor("biasneg", [128, R], F32, kind="ExternalInput")
    iden = nc.dram_tensor("iden", [128, 128], F16, kind="ExternalInput")
    out = nc.dram_tensor("out", [C, H, W], F16, kind="ExternalOutput")

    with tile.TileContext(nc) as tc, ExitStack() as ctx:
        const_pool = ctx.enter_context(tc.tile_pool(name="const", bufs=1))
        hin_pool = ctx.enter_context(tc.tile_pool(name="hin", bufs=6))
        vin_pool = ctx.enter_context(tc.tile_pool(name="vin", bufs=5))
        pd_pool = ctx.enter_context(tc.tile_pool(name="pd", bufs=4))
        # pass 2 of group g is emitted after pass 1 of g+1 (software
        # pipelining), so two groups' worth of pass-1 results are live.
        hacc_pool = ctx.enter_context(tc.tile_pool(name="hacc", bufs=2 * nT + 1))
        st_pool = ctx.enter_context(tc.tile_pool(name="st", bufs=4))
        psf_pool = ctx.enter_context(tc.tile_pool(name="psf", bufs=3, space="PSUM"))

        identity = const_pool.tile([128, 128], F16)
        nc.sync.dma_start(identity[:], iden.ap())
        biasneg_t = const_pool.tile([128, R], F32)
        nc.sync.dma_start(biasneg_t[:], biasneg.ap())
        neg_t = const_pool.tile([128, CP * R], F16)
        nc.gpsimd.memset(neg_t[:], NEG)

        pad_seen = {}

        def set_pads(tile_, tag):
            """Write NEG into the halo pads; once per physical buffer if
            PADS_ONCE (pool buffers rotate round-robin, pads are never
            overwritten afterwards)."""
            n = pad_seen.get(tag, 0)
            if PADS_ONCE and n >= 6:
                return
            pad_seen[tag] = n + 1
            v = tile_[:].rearrange("p (s c) -> p s c", s=CP)
            src = neg_t[:].rearrange("p (s c) -> p s c", s=CP)
            nc.scalar.copy(v[:, :, 0:R], src)
            nc.scalar.copy(v[:, :, SL - R : SL], src)

        def alloc_pd(src3, dram_src=None, skip_center=False):
            """Allocate the 6-slot scratch and DMA the center tap into slot 5
            while other engines are busy. Pass 1 sources it straight from
            DRAM (independent of the halo load); pass 2 copies SBUF->SBUF
            from the transposed tile."""
            pd = pd_pool.tile([128, 6 * CP * L], F16, tag="pd")
            pd_v = pd[:].rearrange("p (d s c) -> p d s c", d=6, s=CP)
            if not skip_center:
                nc.sync.dma_start(
                    pd_v[:, R],
                    dram_src if dram_src is not None else src3[:, :, R : R + L],
                )
            return pd_v

        def dilate(eng, out_v, src3, pd_v, center_src=None, split_out=None):
            _dilate_dve(nc, pd_v, out_v, src3, biasneg_t, bias_vals,
                        center_src=center_src, split_out=split_out)

        def pass1(g):
            haccs = []
            for t in range(nT):
                ht = hin_pool.tile([128, CP * SL], F16, tag="hin")
                set_pads(ht, "hin")
                src = imt.ap()[
                    g * CP : (g + 1) * CP, t * 128 : (t + 1) * 128, :
                ].rearrange("c w h -> w c h")
                hv = ht[:].rearrange("p (s c) -> p s c", s=CP)
                nc.sync.dma_start(hv[:, :, R : R + L], src)
                first = g == 0 and t == 0
                pd_v = alloc_pd(hv, dram_src=src, skip_center=first)
                acc = hacc_pool.tile([128, CP * L], F16, tag="hacc")
                accv = acc[:].rearrange("p (s c) -> p s c", s=CP)
                dilate(_gp_tile(g, t), accv, hv, pd_v,
                       center_src=hv[:, :, R : R + L] if first else None)
                haccs.append(acc)
            return haccs

        def pass2(g, haccs):
            for t2 in range(nT):
                vt = vin_pool.tile([128, CP * SL], F16, tag="vin")
                set_pads(vt, "vin")
                vv = vt[:].rearrange("p (s c) -> p s c", s=CP)
                pt = psf_pool.tile([128, CP * L], F16, tag="psf")
                pt_v = pt[:].rearrange("p (s c) -> p s c", s=CP)
                for ci in range(CP):
                    for t1 in range(nT):
                        nc.tensor.transpose(
                            pt_v[:, ci, t1 * 128 : (t1 + 1) * 128],
                            haccs[t1][:, ci * L + t2 * 128 : ci * L + (t2 + 1) * 128],
                            identity[:],
                        )
                nc.scalar.copy(vv[:, :, R : R + L], pt_v)
                pd_v = alloc_pd(vv)
                stt = st_pool.tile([128, CP * L], F16, tag="st")
                stv = stt[:].rearrange("p (s c) -> p s c", s=CP)
                dst = out.ap()[
                    g * CP : (g + 1) * CP, t2 * 128 : (t2 + 1) * 128, :
                ].rearrange("c h w -> h c w")
                if g == nG - 1 and t2 == nT - 1:
                    def store_half(c0, c1):
                        nc.sync.dma_start(dst[:, c0:c1], stv[:, c0:c1])
                    dilate(_gp_tile(g, nT + t2), stv, vv, pd_v,
                           split_out=store_half)
                else:
                    dilate(_gp_tile(g, nT + t2), stv, vv, pd_v)
                    nc.sync.dma_start(dst, stv)

        for _rep in range(reps):
            # Software pipeline: pass 2 of group g is emitted after pass 1 of
            # group g+1, so every engine always has two groups of independent
            # work in its instruction window.
            prev = None
            for g in range(nG):
                haccs = pass1(g)
                if prev is not None:
                    pass2(prev[0], prev[1])
                prev = (g, haccs)
            pass2(prev[0], prev[1])

    nc.compile()
    return nc


_NC_CACHE = {}


def _get_nc(bias_vals=None):
    if bias_vals is None:
        bias_vals = next(iter(_NC_CACHE))
    if bias_vals not in _NC_CACHE:
        _NC_CACHE[bias_vals] = build_nc(bias_vals)
    return _NC_CACHE[bias_vals]


def _bias_vals(se_coef, se):
    se = np.asarray(se, dtype=np.float32)
    se_coef = np.asarray(se_coef, dtype=np.float32)
    bias11 = (se_coef * se[:, 0]).astype(np.float32)  # same fp32 op as reference
    return tuple(float(x) for x in bias11[R + 1 : 2 * R + 1])


def _make_in_maps(im, se_coef, se):
    im = np.asarray(im, dtype=np.float32)
    bias_vals = _bias_vals(se_coef, se)
    biasneg = np.ascontiguousarray(
        np.broadcast_to(-np.asarray(bias_vals, dtype=np.float32), (128, R))
    )
    iden = np.eye(128, dtype=np.float16)
    imt = np.ascontiguousarray(np.swapaxes(im, 2, 3)).astype(np.float16)
    return [
        {"imt": imt[b], "biasneg": biasneg, "iden": iden} for b in range(im.shape[0])
    ]


def kernel(im, se_coef, se):
    bias_vals = _bias_vals(se_coef, se)
    nc = _get_nc(bias_vals)
    in_maps = _make_in_maps(im, se_coef, se)
    res = run_bass_kernel_spmd(nc, in_maps, core_ids=list(range(N_CORES)))
    out = np.stack([res.results[b]["out"] for b in range(N_CORES)], axis=0)
    return out.astype(np.float32)


# revision 32
# speedup vs baseline: 1.0069x; 1.0047x over previous
"""Trainium2 Bass kernel: separable parabolic morphological dilation (11-tap).

nn_Dilation2dSingle: im [8, 32, 512, 512] f32, se_coef scalar, se [11, 1].
    bias[k] = se_coef * se[k, 0]           (parabolic, symmetric, bias[5] = 0)
    out = vdilate(hdilate(im)) with NEG=-10000 padding.

Strategy (v2, fp16):
  * Host converts im to fp16 and pre-transposes each sample to [C, W, H], so
    the kernel runs the vertical pass first (along the free axis), does ONE
    on-device transpose (PE identity-matmul into PSUM, ACT copy back), runs
    the horizontal pass, and stores [C, H, W] directly -- no second transpose.
  * Each 1D pass is computed in unfused form to exploit DVE perf modes:
      - one batched tensor_max over a strided 4D AP computes all 5 symmetric
        pair maxima p_d = max(x[i-d], x[i+d]) in a single 2x-mode instruction
      - the 5 bias subtracts q_d = p_d - b_d are single-src ops: 4x mode on
        DVE (tensor_scalar) or offloaded to ACT (Identity activation with a
        per-partition bias AP)
      - a 4-instruction max tree combines q_1..q_5 and the center tap
    This is ~6.25 DVE-cyc/elem/pass vs 15 for the naive fused form
    (scalar_tensor_tensor has no 2x uop; tensor_max does).
  * GPSIMD cannot help: on TRN2 walrus only accepts float add/sub/mult (no
    max) TensorTensor on the Pool engine, and InstPool must run on DVE; the
    relu-max decomposition (GP sub + ACT relu + GP add) loses to its
    cross-engine latency. So DVE carries all max work and ACT the subtracts.
  * Bias values are compiled in as immediates/constants derived from the
    runtime se_coef (the module cache is keyed on them).

Sharding: pure data-parallel over batch (8 cores x 1 batch each).
"""

from contextlib import ExitStack

import numpy as np

import concourse.bacc as bacc
import concourse.mybir as mybir
import concourse.tile as tile
from concourse.bass_utils import run_bass_kernel_spmd

F16 = mybir.dt.float16
F32 = mybir.dt.float32
NEG = -10000.0
R = 5  # dilation radius (window 11)

# Hardcoded problem shape (per spec).
B, C, H, W = 8, 32, 512, 512
N_CORES = 8
CP = 4  # channels packed per instruction group
L = 512  # pass length (H == W == 512)
SL = L + 2 * R
nT = L // 128  # partition tiles per pass
nG = C // CP  # channel groups

def _gp_tile(g, k):
    """Tile mode for tile k (0-3 pass 1, 4-7 pass 2) of group g. "dve" is
    the only profitable mode on TRN2 (see header); "assist"/"assist_dsub"
    (relu-max via GPSIMD+ACT) are kept for experimentation but lose to
    cross-engine latency."""
    return "dve"
ACT_SUBS = True  # bias subtracts on ACT (True) or DVE 4x tensor_scalar
PADS_ONCE = True  # write NEG halos once per pool buffer instead of per tile


def _win_pair_aps(src3, pd_view, nd=R):
    """Build the (left, right) strided APs covering the first nd shifted taps.

    src3: AP [128, CP, SL]; returns 4D APs [128, nd, CP, L] where dim 1 walks
    d=1..nd via stride -1/+1 starting at offset R-1 / R+1.
    """
    left = src3[:, :, R - 1 : R - 1 + L].unsqueeze(1).copy()
    left.ap[1] = [-1, nd]
    right = src3[:, :, R + 1 : R + 1 + L].unsqueeze(1).copy()
    right.ap[1] = [1, nd]
    return left, right


def _dilate_dve(nc, pd_v, out_v, src3, biasneg_t, bias_vals,
                center_src=None, split_out=None):
    """Unfused pair/sub/tree dilation on DVE (+ACT for the subtracts).

    pd_v: [128, 6, CP, L] scratch whose slot 5 already holds the center tap
    (written by an off-engine SBUF->SBUF DMA at input-ready time). The
    6-leaf max tree then runs as a 3-instruction in-place block cascade:
    slots (0,1),(2,3),(4,5) -> slots 0,1,2; (0,1) -> 0; (0,2) -> out.
    In-place is safe: each block's reads finish a full CP*L-element stride
    before any later block overwrites that region."""
    left, right = _win_pair_aps(src3, pd_v)
    nc.vector.tensor_tensor(
        pd_v[:, 0:R], left, right, op=mybir.AluOpType.max
    )
    for d in range(1, R + 1):
        q = pd_v[:, d - 1]
        nc.scalar.activation(
            q, q, mybir.ActivationFunctionType.Identity,
            bias=biasneg_t[:, d - 1 : d],
        )
    if center_src is not None:
        # first-tile variant: no center DMA (it would serialize behind the
        # very first load); classic 4-instruction tree reading the center
        # from the input tile instead.
        sa = pd_v[:, 0:1].copy()
        sa.ap[1] = [2 * CP * L, 2]  # slots 0, 2
        sb = pd_v[:, 1:2].copy()
        sb.ap[1] = [2 * CP * L, 2]  # slots 1, 3
        nc.vector.tensor_tensor(pd_v[:, 0:2], sa, sb, op=mybir.AluOpType.max)
        nc.vector.tensor_tensor(pd_v[:, 0], pd_v[:, 0], pd_v[:, 1],
                                op=mybir.AluOpType.max)
        nc.vector.tensor_tensor(pd_v[:, 1], pd_v[:, 4], center_src,
                                op=mybir.AluOpType.max)
        nc.vector.tensor_tensor(out_v, pd_v[:, 0], pd_v[:, 1],
                                op=mybir.AluOpType.max)
        return
    # Fused levels 1+2 of the 6-leaf tree as one 4-block in-place cascade:
    # block b computes max(slot[1+2b], slot[2b]) -> slot[7-b], i.e.
    #   (q2,q1)->7, (q4,q3)->6, (c,q5)->5, (out7,out6)->4
    # All three APs are affine (src stride 2*CP*L, out stride -CP*L) and
    # each block's reads complete a full CP*L-element stride before any
    # later block overwrites that region, so the overlap is stream-safe.
    sa = pd_v[:, 1:2].copy()
    sa.ap[1] = [2 * CP * L, 4]  # slots 1, 3, 5, 7
    sb = pd_v[:, 0:1].copy()
    sb.ap[1] = [2 * CP * L, 4]  # slots 0, 2, 4, 6
    so = pd_v[:, 7:8].copy()
    so.ap[1] = [-CP * L, 4]  # slots 7, 6, 5, 4
    nc.vector.tensor_tensor(so, sa, sb, op=mybir.AluOpType.max)
    if split_out is None:
        nc.vector.tensor_tensor(out_v, pd_v[:, 4], pd_v[:, 5],
                                op=mybir.AluOpType.max)
    else:
        # final-tile variant: emit the last max in two channel halves with
        # the store callback between them, so half the store overlaps the
        # second half of the compute (shrinks the pipeline tail).
        h = CP // 2
        nc.vector.tensor_tensor(out_v[:, 0:h], pd_v[:, 4, 0:h],
                                pd_v[:, 5, 0:h], op=mybir.AluOpType.max)
        split_out(0, h)
        nc.vector.tensor_tensor(out_v[:, h:CP], pd_v[:, 4, h:CP],
                                pd_v[:, 5, h:CP], op=mybir.AluOpType.max)
        split_out(h, CP)


def build_nc(bias_vals, reps=1):
    """Build the per-core Bass module. bias_vals: tuple of 5 floats (b_1..b_5).

    reps > 1 repeats the whole pipeline (same output) -- used only for
    differential timing experiments."""
    nc = bacc.Bacc("TRN2", target_bir_lowering=False, debug=False)
    imt = nc.dram_tensor("imt", [C, W, H], F16, kind="ExternalInput")
    biasneg = nc.dram_tensor("biasneg", [128, R], F32, kind="ExternalInput")
    iden = nc.dram_tensor("iden", [128, 128], F16, kind="ExternalInput")
    out = nc.dram_tensor("out", [C, H, W], F16, kind="ExternalOutput")

    with tile.TileContext(nc) as tc, ExitStack() as ctx:
        const_pool = ctx.enter_context(tc.tile_pool(name="const", bufs=1))
        hin_pool = ctx.enter_context(tc.tile_pool(name="hin", bufs=6))
        vin_pool = ctx.enter_context(tc.tile_pool(name="vin", bufs=5))
        pd_pool = ctx.enter_context(tc.tile_pool(name="pd", bufs=3))
        # pass 2 of group g is emitted after pass 1 of g+1 (software
        # pipelining), so two groups' worth of pass-1 results are live.
        hacc_pool = ctx.enter_context(tc.tile_pool(name="hacc", bufs=2 * nT + 1))
        st_pool = ctx.enter_context(tc.tile_pool(name="st", bufs=4))
        psf_pool = ctx.enter_context(tc.tile_pool(name="psf", bufs=3, space="PSUM"))

        identity = const_pool.tile([128, 128], F16)
        nc.sync.dma_start(identity[:], iden.ap())
        biasneg_t = const_pool.tile([128, R], F32)
        nc.sync.dma_start(biasneg_t[:], biasneg.ap())
        neg_t = const_pool.tile([128, CP * R], F16)
        nc.gpsimd.memset(neg_t[:], NEG)

        pad_seen = {}

        def set_pads(tile_, tag):
            """Write NEG into the halo pads; once per physical buffer if
            PADS_ONCE (pool buffers rotate round-robin, pads are never
            overwritten afterwards)."""
            n = pad_seen.get(tag, 0)
            if PADS_ONCE and n >= 6:
                return
            pad_seen[tag] = n + 1
            v = tile_[:].rearrange("p (s c) -> p s c", s=CP)
            src = neg_t[:].rearrange("p (s c) -> p s c", s=CP)
            nc.scalar.copy(v[:, :, 0:R], src)
            nc.scalar.copy(v[:, :, SL - R : SL], src)

        def alloc_pd(src3, dram_src=None, skip_center=False):
            """Allocate the 6-slot scratch and DMA the center tap into slot 5
            while other engines are busy. Pass 1 sources it straight from
            DRAM (independent of the halo load); pass 2 copies SBUF->SBUF
            from the transposed tile."""
            pd = pd_pool.tile([128, 8 * CP * L], F16, tag="pd")
            pd_v = pd[:].rearrange("p (d s c) -> p d s c", d=8, s=CP)
            if not skip_center:
                nc.sync.dma_start(
                    pd_v[:, R],
                    dram_src if dram_src is not None else src3[:, :, R : R + L],
                )
            return pd_v

        def dilate(eng, out_v, src3, pd_v, center_src=None, split_out=None):
            _dilate_dve(nc, pd_v, out_v, src3, biasneg_t, bias_vals,
                        center_src=center_src, split_out=split_out)

        def pass1(g):
            haccs = []
            for t in range(nT):
                ht = hin_pool.tile([128, CP * SL], F16, tag="hin")
                set_pads(ht, "hin")
                src = imt.ap()[
                    g * CP : (g + 1) * CP, t * 128 : (t + 1) * 128, :
                ].rearrange("c w h -> w c h")
                hv = ht[:].rearrange("p (s c) -> p s c", s=CP)
                nc.sync.dma_start(hv[:, :, R : R + L], src)
                first = g == 0 and t == 0
                pd_v = alloc_pd(hv, dram_src=src, skip_center=first)
                acc = hacc_pool.tile([128, CP * L], F16, tag="hacc")
                accv = acc[:].rearrange("p (s c) -> p s c", s=CP)
                dilate(_gp_tile(g, t), accv, hv, pd_v,
                       center_src=hv[:, :, R : R + L] if first else None)
                haccs.append(acc)
            return haccs

        def pass2(g, haccs):
            for t2 in range(nT):
                vt = vin_pool.tile([128, CP * SL], F16, tag="vin")
                set_pads(vt, "vin")
                vv = vt[:].rearrange("p (s c) -> p s c", s=CP)
                pt = psf_pool.tile([128, CP * L], F16, tag="psf")
                pt_v = pt[:].rearrange("p (s c) -> p s c", s=CP)
                for ci in range(CP):
                    for t1 in range(nT):
                        nc.tensor.transpose(
                            pt_v[:, ci, t1 * 128 : (t1 + 1) * 128],
                            haccs[t1][:, ci * L + t2 * 128 : ci * L + (t2 + 1) * 128],
                            identity[:],
                        )
                nc.scalar.copy(vv[:, :, R : R + L], pt_v)
                pd_v = alloc_pd(vv)
                stt = st_pool.tile([128, CP * L], F16, tag="st")
                stv = stt[:].rearrange("p (s c) -> p s c", s=CP)
                dst = out.ap()[
                    g * CP : (g + 1) * CP, t2 * 128 : (t2 + 1) * 128, :
                ].rearrange("c h w -> h c w")
                if g == nG - 1 and t2 == nT - 1:
                    def store_half(c0, c1):
                        nc.sync.dma_start(dst[:, c0:c1], stv[:, c0:c1])
                    dilate(_gp_tile(g, nT + t2), stv, vv, pd_v,
                           split_out=store_half)
                else:
                    dilate(_gp_tile(g, nT + t2), stv, vv, pd_v)
                    nc.sync.dma_start(dst, stv)

        for _rep in range(reps):
            # Software pipeline: pass 2 of group g is emitted after pass 1 of
            # group g+1, so every engine always has two groups of independent
            # work in its instruction window.
            prev = None
            for g in range(nG):
                haccs = pass1(g)
                if prev is not None:
                    pass2(prev[0], prev[1])
                prev = (g, haccs)
            pass2(prev[0], prev[1])

    nc.compile()
    return nc


_NC_CACHE = {}


def _get_nc(bias_vals=None):
    if bias_vals is None:
        bias_vals = next(iter(_NC_CACHE))
    if bias_vals not in _NC_CACHE:
        _NC_CACHE[bias_vals] = build_nc(bias_vals)
    return _NC_CACHE[bias_vals]


def _bias_vals(se_coef, se):
    se = np.asarray(se, dtype=np.float32)
    se_coef = np.asarray(se_coef, dtype=np.float32)
    bias11 = (se_coef * se[:, 0]).astype(np.float32)  # same fp32 op as reference
    return tuple(float(x) for x in bias11[R + 1 : 2 * R + 1])


def _make_in_maps(im, se_coef, se):
    im = np.asarray(im, dtype=np.float32)
    bias_vals = _bias_vals(se_coef, se)
    biasneg = np.ascontiguousarray(
        np.broadcast_to(-np.asarray(bias_vals, dtype=np.float32), (128, R))
    )
    iden = np.eye(128, dtype=np.float16)
    imt = np.ascontiguousarray(np.swapaxes(im, 2, 3)).astype(np.float16)
    return [
        {"imt": imt[b], "biasneg": biasneg, "iden": iden} for b in range(im.shape[0])
    ]


def kernel(im, se_coef, se):
    bias_vals = _bias_vals(se_coef, se)
    nc = _get_nc(bias_vals)
    in_maps = _make_in_maps(im, se_coef, se)
    res = run_bass_kernel_spmd(nc, in_maps, core_ids=list(range(N_CORES)))
    out = np.stack([res.results[b]["out"] for b in range(N_CORES)], axis=0)
    return out.astype(np.float32)


# revision 34
# speedup vs baseline: 1.0110x; 1.0040x over previous
"""Trainium2 Bass kernel: separable parabolic morphological dilation (11-tap).

nn_Dilation2dSingle: im [8, 32, 512, 512] f32, se_coef scalar, se [11, 1].
    bias[k] = se_coef * se[k, 0]           (parabolic, symmetric, bias[5] = 0)
    out = vdilate(hdilate(im)) with NEG=-10000 padding.

Strategy (v2, fp16):
  * Host converts im to fp16 and pre-transposes each sample to [C, W, H], so
    the kernel runs the vertical pass first (along the free axis), does ONE
    on-device transpose (PE identity-matmul into PSUM, ACT copy back), runs
    the horizontal pass, and stores [C, H, W] directly -- no second transpose.
  * Each 1D pass is computed in unfused form to exploit DVE perf modes:
      - one batched tensor_max over a strided 4D AP computes all 5 symmetric
        pair maxima p_d = max(x[i-d], x[i+d]) in a single 2x-mode instruction
      - the 5 bias subtracts q_d = p_d - b_d are single-src ops: 4x mode on
        DVE (tensor_scalar) or offloaded to ACT (Identity activation with a
        per-partition bias AP)
      - a 4-instruction max tree combines q_1..q_5 and the center tap
    This is ~6.25 DVE-cyc/elem/pass vs 15 for the naive fused form
    (scalar_tensor_tensor has no 2x uop; tensor_max does).
  * GPSIMD cannot help: on TRN2 walrus only accepts float add/sub/mult (no
    max) TensorTensor on the Pool engine, and InstPool must run on DVE; the
    relu-max decomposition (GP sub + ACT relu + GP add) loses to its
    cross-engine latency. So DVE carries all max work and ACT the subtracts.
  * Bias values are compiled in as immediates/constants derived from the
    runtime se_coef (the module cache is keyed on them).

Sharding: pure data-parallel over batch (8 cores x 1 batch each).
"""

from contextlib import ExitStack

import numpy as np

import concourse.bacc as bacc
import concourse.mybir as mybir
import concourse.tile as tile
from concourse.bass_utils import run_bass_kernel_spmd

F16 = mybir.dt.float16
F32 = mybir.dt.float32
NEG = -10000.0
R = 5  # dilation radius (window 11)

# Hardcoded problem shape (per spec).
B, C, H, W = 8, 32, 512, 512
N_CORES = 8
CP = 4  # channels packed per instruction group
L = 512  # pass length (H == W == 512)
SL = L + 2 * R
nT = L // 128  # partition tiles per pass
nG = C // CP  # channel groups

def _gp_tile(g, k):
    """Tile mode for tile k (0-3 pass 1, 4-7 pass 2) of group g. "dve" is
    the only profitable mode on TRN2 (see header); "assist"/"assist_dsub"
    (relu-max via GPSIMD+ACT) are kept for experimentation but lose to
    cross-engine latency."""
    return "dve"
ACT_SUBS = True  # bias subtracts on ACT (True) or DVE 4x tensor_scalar
PADS_ONCE = True  # write NEG halos once per pool buffer instead of per tile


def _win_pair_aps(src3, pd_view, nd=R):
    """Build the (left, right) strided APs covering the first nd shifted taps.

    src3: AP [128, CP, SL]; returns 4D APs [128, nd, CP, L] where dim 1 walks
    d=1..nd via stride -1/+1 starting at offset R-1 / R+1.
    """
    left = src3[:, :, R - 1 : R - 1 + L].unsqueeze(1).copy()
    left.ap[1] = [-1, nd]
    right = src3[:, :, R + 1 : R + 1 + L].unsqueeze(1).copy()
    right.ap[1] = [1, nd]
    return left, right


def _dilate_dve(nc, pd_v, out_v, src3, biasneg_t, bias_vals,
                center_src=None, split_out=None):
    """Unfused pair/sub/tree dilation on DVE (+ACT for the subtracts).

    pd_v: [128, 6, CP, L] scratch whose slot 5 already holds the center tap
    (written by an off-engine SBUF->SBUF DMA at input-ready time). The
    6-leaf max tree then runs as a 3-instruction in-place block cascade:
    slots (0,1),(2,3),(4,5) -> slots 0,1,2; (0,1) -> 0; (0,2) -> out.
    In-place is safe: each block's reads finish a full CP*L-element stride
    before any later block overwrites that region."""
    left, right = _win_pair_aps(src3, pd_v)
    nc.vector.tensor_tensor(
        pd_v[:, 0:R], left, right, op=mybir.AluOpType.max
    )
    for d in range(1, R + 1):
        q = pd_v[:, d - 1]
        nc.scalar.activation(
            q, q, mybir.ActivationFunctionType.Identity,
            bias=biasneg_t[:, d - 1 : d],
        )
    if center_src is not None:
        # edge-tile variant: no center DMA (it would serialize behind the
        # very first load / the final stores); classic 4-instruction tree
        # reading the center from the input tile instead.
        sa = pd_v[:, 0:1].copy()
        sa.ap[1] = [2 * CP * L, 2]  # slots 0, 2
        sb = pd_v[:, 1:2].copy()
        sb.ap[1] = [2 * CP * L, 2]  # slots 1, 3
        nc.vector.tensor_tensor(pd_v[:, 0:2], sa, sb, op=mybir.AluOpType.max)
        nc.vector.tensor_tensor(pd_v[:, 0], pd_v[:, 0], pd_v[:, 1],
                                op=mybir.AluOpType.max)
        nc.vector.tensor_tensor(pd_v[:, 1], pd_v[:, 4], center_src,
                                op=mybir.AluOpType.max)
        if split_out is None:
            nc.vector.tensor_tensor(out_v, pd_v[:, 0], pd_v[:, 1],
                                    op=mybir.AluOpType.max)
        else:
            h = CP // 2
            nc.vector.tensor_tensor(out_v[:, 0:h], pd_v[:, 0, 0:h],
                                    pd_v[:, 1, 0:h], op=mybir.AluOpType.max)
            split_out(0, h)
            nc.vector.tensor_tensor(out_v[:, h:CP], pd_v[:, 0, h:CP],
                                    pd_v[:, 1, h:CP], op=mybir.AluOpType.max)
            split_out(h, CP)
        return
    # Fused levels 1+2 of the 6-leaf tree as one 4-block in-place cascade:
    # block b computes max(slot[1+2b], slot[2b]) -> slot[7-b], i.e.
    #   (q2,q1)->7, (q4,q3)->6, (c,q5)->5, (out7,out6)->4
    # All three APs are affine (src stride 2*CP*L, out stride -CP*L) and
    # each block's reads complete a full CP*L-element stride before any
    # later block overwrites that region, so the overlap is stream-safe.
    sa = pd_v[:, 1:2].copy()
    sa.ap[1] = [2 * CP * L, 4]  # slots 1, 3, 5, 7
    sb = pd_v[:, 0:1].copy()
    sb.ap[1] = [2 * CP * L, 4]  # slots 0, 2, 4, 6
    so = pd_v[:, 7:8].copy()
    so.ap[1] = [-CP * L, 4]  # slots 7, 6, 5, 4
    nc.vector.tensor_tensor(so, sa, sb, op=mybir.AluOpType.max)
    if split_out is None:
        nc.vector.tensor_tensor(out_v, pd_v[:, 4], pd_v[:, 5],
                                op=mybir.AluOpType.max)
    else:
        # final-tile variant: emit the last max in two channel halves with
        # the store callback between them, so half the store overlaps the
        # second half of the compute (shrinks the pipeline tail).
        h = CP // 2
        nc.vector.tensor_tensor(out_v[:, 0:h], pd_v[:, 4, 0:h],
                                pd_v[:, 5, 0:h], op=mybir.AluOpType.max)
        split_out(0, h)
        nc.vector.tensor_tensor(out_v[:, h:CP], pd_v[:, 4, h:CP],
                                pd_v[:, 5, h:CP], op=mybir.AluOpType.max)
        split_out(h, CP)


def build_nc(bias_vals, reps=1):
    """Build the per-core Bass module. bias_vals: tuple of 5 floats (b_1..b_5).

    reps > 1 repeats the whole pipeline (same output) -- used only for
    differential timing experiments."""
    nc = bacc.Bacc("TRN2", target_bir_lowering=False, debug=False)
    imt = nc.dram_tensor("imt", [C, W, H], F16, kind="ExternalInput")
    biasneg = nc.dram_tensor("biasneg", [128, R], F32, kind="ExternalInput")
    iden = nc.dram_tensor("iden", [128, 128], F16, kind="ExternalInput")
    out = nc.dram_tensor("out", [C, H, W], F16, kind="ExternalOutput")

    with tile.TileContext(nc) as tc, ExitStack() as ctx:
        const_pool = ctx.enter_context(tc.tile_pool(name="const", bufs=1))
        hin_pool = ctx.enter_context(tc.tile_pool(name="hin", bufs=6))
        vin_pool = ctx.enter_context(tc.tile_pool(name="vin", bufs=5))
        pd_pool = ctx.enter_context(tc.tile_pool(name="pd", bufs=3))
        # pass 2 of group g is emitted after pass 1 of g+1 (software
        # pipelining), so two groups' worth of pass-1 results are live.
        hacc_pool = ctx.enter_context(tc.tile_pool(name="hacc", bufs=2 * nT + 1))
        st_pool = ctx.enter_context(tc.tile_pool(name="st", bufs=4))
        psf_pool = ctx.enter_context(tc.tile_pool(name="psf", bufs=3, space="PSUM"))

        identity = const_pool.tile([128, 128], F16)
        biasneg_t = const_pool.tile([128, R], F32)
        neg_t = const_pool.tile([128, CP * R], F16)
        nc.gpsimd.memset(neg_t[:], NEG)
        # identity/biasneg DMAs are deferred until the first image load is
        # in flight so they don't occupy HWDGE/DMA ahead of it (identity is
        # not needed until pass 2, biasneg not until the first subtracts).
        deferred_consts = [
            lambda: nc.sync.dma_start(identity[:], iden.ap()),
            lambda: nc.sync.dma_start(biasneg_t[:], biasneg.ap()),
        ]

        pad_seen = {}

        def set_pads(tile_, tag):
            """Write NEG into the halo pads; once per physical buffer if
            PADS_ONCE (pool buffers rotate round-robin, pads are never
            overwritten afterwards)."""
            n = pad_seen.get(tag, 0)
            if PADS_ONCE and n >= 6:
                return
            pad_seen[tag] = n + 1
            v = tile_[:].rearrange("p (s c) -> p s c", s=CP)
            src = neg_t[:].rearrange("p (s c) -> p s c", s=CP)
            nc.scalar.copy(v[:, :, 0:R], src)
            nc.scalar.copy(v[:, :, SL - R : SL], src)

        def alloc_pd(src3, dram_src=None, skip_center=False):
            """Allocate the 6-slot scratch and DMA the center tap into slot 5
            while other engines are busy. Pass 1 sources it straight from
            DRAM (independent of the halo load); pass 2 copies SBUF->SBUF
            from the transposed tile."""
            pd = pd_pool.tile([128, 8 * CP * L], F16, tag="pd")
            pd_v = pd[:].rearrange("p (d s c) -> p d s c", d=8, s=CP)
            if not skip_center:
                nc.sync.dma_start(
                    pd_v[:, R],
                    dram_src if dram_src is not None else src3[:, :, R : R + L],
                )
            return pd_v

        def dilate(eng, out_v, src3, pd_v, center_src=None, split_out=None):
            _dilate_dve(nc, pd_v, out_v, src3, biasneg_t, bias_vals,
                        center_src=center_src, split_out=split_out)

        def pass1(g):
            haccs = []
            for t in range(nT):
                ht = hin_pool.tile([128, CP * SL], F16, tag="hin")
                set_pads(ht, "hin")
                src = imt.ap()[
                    g * CP : (g + 1) * CP, t * 128 : (t + 1) * 128, :
                ].rearrange("c w h -> w c h")
                hv = ht[:].rearrange("p (s c) -> p s c", s=CP)
                nc.sync.dma_start(hv[:, :, R : R + L], src)
                while deferred_consts:
                    deferred_consts.pop()()
                first = g == 0 and t == 0
                pd_v = alloc_pd(hv, dram_src=src, skip_center=first)
                acc = hacc_pool.tile([128, CP * L], F16, tag="hacc")
                accv = acc[:].rearrange("p (s c) -> p s c", s=CP)
                dilate(_gp_tile(g, t), accv, hv, pd_v,
                       center_src=hv[:, :, R : R + L] if first else None)
                haccs.append(acc)
            return haccs

        def pass2(g, haccs):
            for t2 in range(nT):
                vt = vin_pool.tile([128, CP * SL], F16, tag="vin")
                set_pads(vt, "vin")
                vv = vt[:].rearrange("p (s c) -> p s c", s=CP)
                pt = psf_pool.tile([128, CP * L], F16, tag="psf")
                pt_v = pt[:].rearrange("p (s c) -> p s c", s=CP)
                for ci in range(CP):
                    for t1 in range(nT):
                        nc.tensor.transpose(
                            pt_v[:, ci, t1 * 128 : (t1 + 1) * 128],
                            haccs[t1][:, ci * L + t2 * 128 : ci * L + (t2 + 1) * 128],
                            identity[:],
                        )
                nc.scalar.copy(vv[:, :, R : R + L], pt_v)
                pd_v = alloc_pd(
                    vv, skip_center=(g == nG - 1 and t2 == nT - 1)
                )
                stt = st_pool.tile([128, CP * L], F16, tag="st")
                stv = stt[:].rearrange("p (s c) -> p s c", s=CP)
                dst = out.ap()[
                    g * CP : (g + 1) * CP, t2 * 128 : (t2 + 1) * 128, :
                ].rearrange("c h w -> h c w")
                if g == nG - 1 and t2 == nT - 1:
                    def store_half(c0, c1):
                        nc.sync.dma_start(dst[:, c0:c1], stv[:, c0:c1])
                    dilate(_gp_tile(g, nT + t2), stv, vv, pd_v,
                           center_src=vv[:, :, R : R + L],
                           split_out=store_half)
                else:
                    dilate(_gp_tile(g, nT + t2), stv, vv, pd_v)
                    nc.sync.dma_start(dst, stv)

        for _rep in range(reps):
            # Software pipeline: pass 2 of group g is emitted after pass 1 of
            # group g+1, so every engine always has two groups of independent
            # work in its instruction window.
            prev = None
            for g in range(nG):
                haccs = pass1(g)
                if prev is not None:
                    pass2(prev[0], prev[1])
                prev = (g, haccs)
            pass2(prev[0], prev[1])

    nc.compile()
    return nc


_NC_CACHE = {}


def _get_nc(bias_vals=None):
    if bias_vals is None:
        bias_vals = next(iter(_NC_CACHE))
    if bias_vals not in _NC_CACHE:
        _NC_CACHE[bias_vals] = build_nc(bias_vals)
    return _NC_CACHE[bias_vals]


def _bias_vals(se_coef, se):
    se = np.asarray(se, dtype=np.float32)
    se_coef = np.asarray(se_coef, dtype=np.float32)
    bias11 = (se_coef * se[:, 0]).astype(np.float32)  # same fp32 op as reference
    return tuple(float(x) for x in bias11[R + 1 : 2 * R + 1])


def _make_in_maps(im, se_coef, se):
    im = np.asarray(im, dtype=np.float32)
    bias_vals = _bias_vals(se_coef, se)
    biasneg = np.ascontiguousarray(
        np.broadcast_to(-np.asarray(bias_vals, dtype=np.float32), (128, R))
    )
    iden = np.eye(128, dtype=np.float16)
    imt = np.ascontiguousarray(np.swapaxes(im, 2, 3)).astype(np.float16)
    return [
        {"imt": imt[b], "biasneg": biasneg, "iden": iden} for b in range(im.shape[0])
    ]


def kernel(im, se_coef, se):
    bias_vals = _bias_vals(se_coef, se)
    nc = _get_nc(bias_vals)
    in_maps = _make_in_maps(im, se_coef, se)
    res = run_bass_kernel_spmd(nc, in_maps, core_ids=list(range(N_CORES)))
    out = np.stack([res.results[b]["out"] for b in range(N_CORES)], axis=0)
    return out.astype(np.float32)


# revision 36
# speedup vs baseline: 1.0112x; 1.0002x over previous
"""Trainium2 Bass kernel: separable parabolic morphological dilation (11-tap).

nn_Dilation2dSingle: im [8, 32, 512, 512] f32, se_coef scalar, se [11, 1].
    bias[k] = se_coef * se[k, 0]           (parabolic, symmetric, bias[5] = 0)
    out = vdilate(hdilate(im)) with NEG=-10000 padding.

Strategy (v2, fp16):
  * Host converts im to fp16 and pre-transposes each sample to [C, W, H], so
    the kernel runs the vertical pass first (along the free axis), does ONE
    on-device transpose (PE identity-matmul into PSUM, ACT copy back), runs
    the horizontal pass, and stores [C, H, W] directly -- no second transpose.
  * Each 1D pass is computed in unfused form to exploit DVE perf modes:
      - one batched tensor_max over a strided 4D AP computes all 5 symmetric
        pair maxima p_d = max(x[i-d], x[i+d]) in a single 2x-mode instruction
      - the 5 bias subtracts q_d = p_d - b_d are single-src ops: 4x mode on
        DVE (tensor_scalar) or offloaded to ACT (Identity activation with a
        per-partition bias AP)
      - a 4-instruction max tree combines q_1..q_5 and the center tap
    This is ~6.25 DVE-cyc/elem/pass vs 15 for the naive fused form
    (scalar_tensor_tensor has no 2x uop; tensor_max does).
  * GPSIMD cannot help: on TRN2 walrus only accepts float add/sub/mult (no
    max) TensorTensor on the Pool engine, and InstPool must run on DVE; the
    relu-max decomposition (GP sub + ACT relu + GP add) loses to its
    cross-engine latency. So DVE carries all max work and ACT the subtracts.
  * Bias values are compiled in as immediates/constants derived from the
    runtime se_coef (the module cache is keyed on them).

Sharding: pure data-parallel over batch (8 cores x 1 batch each).
"""

from contextlib import ExitStack

import numpy as np

import concourse.bacc as bacc
import concourse.mybir as mybir
import concourse.tile as tile
from concourse.bass_utils import run_bass_kernel_spmd

F16 = mybir.dt.float16
F32 = mybir.dt.float32
NEG = -10000.0
R = 5  # dilation radius (window 11)

# Hardcoded problem shape (per spec).
B, C, H, W = 8, 32, 512, 512
N_CORES = 8
CP = 4  # channels packed per instruction group
L = 512  # pass length (H == W == 512)
SL = L + 2 * R
nT = L // 128  # partition tiles per pass
nG = C // CP  # channel groups

def _gp_tile(g, k):
    """Tile mode for tile k (0-3 pass 1, 4-7 pass 2) of group g. "dve" is
    the only profitable mode on TRN2 (see header); "assist"/"assist_dsub"
    (relu-max via GPSIMD+ACT) are kept for experimentation but lose to
    cross-engine latency."""
    return "dve"
ACT_SUBS = True  # bias subtracts on ACT (True) or DVE 4x tensor_scalar
PADS_ONCE = True  # write NEG halos once per pool buffer instead of per tile


def _win_pair_aps(src3, pd_view, nd=R):
    """Build the (left, right) strided APs covering the first nd shifted taps.

    src3: AP [128, CP, SL]; returns 4D APs [128, nd, CP, L] where dim 1 walks
    d=1..nd via stride -1/+1 starting at offset R-1 / R+1.
    """
    left = src3[:, :, R - 1 : R - 1 + L].unsqueeze(1).copy()
    left.ap[1] = [-1, nd]
    right = src3[:, :, R + 1 : R + 1 + L].unsqueeze(1).copy()
    right.ap[1] = [1, nd]
    return left, right


def _dilate_dve(nc, pd_v, out_v, src3, biasneg_t, bias_vals,
                center_src=None, split_out=None):
    """Unfused pair/sub/tree dilation on DVE (+ACT for the subtracts).

    pd_v: [128, 6, CP, L] scratch whose slot 5 already holds the center tap
    (written by an off-engine SBUF->SBUF DMA at input-ready time). The
    6-leaf max tree then runs as a 3-instruction in-place block cascade:
    slots (0,1),(2,3),(4,5) -> slots 0,1,2; (0,1) -> 0; (0,2) -> out.
    In-place is safe: each block's reads finish a full CP*L-element stride
    before any later block overwrites that region."""
    left, right = _win_pair_aps(src3, pd_v)
    nc.vector.tensor_tensor(
        pd_v[:, 0:R], left, right, op=mybir.AluOpType.max
    )
    for d in range(1, R + 1):
        q = pd_v[:, d - 1]
        nc.scalar.activation(
            q, q, mybir.ActivationFunctionType.Identity,
            bias=biasneg_t[:, d - 1 : d],
        )
    if center_src is not None:
        # edge-tile variant: no center DMA (it would serialize behind the
        # very first load / the final stores); classic 4-instruction tree
        # reading the center from the input tile instead.
        sa = pd_v[:, 0:1].copy()
        sa.ap[1] = [2 * CP * L, 2]  # slots 0, 2
        sb = pd_v[:, 1:2].copy()
        sb.ap[1] = [2 * CP * L, 2]  # slots 1, 3
        nc.vector.tensor_tensor(pd_v[:, 0:2], sa, sb, op=mybir.AluOpType.max)
        nc.vector.tensor_tensor(pd_v[:, 0], pd_v[:, 0], pd_v[:, 1],
                                op=mybir.AluOpType.max)
        nc.vector.tensor_tensor(pd_v[:, 1], pd_v[:, 4], center_src,
                                op=mybir.AluOpType.max)
        if split_out is None:
            nc.vector.tensor_tensor(out_v, pd_v[:, 0], pd_v[:, 1],
                                    op=mybir.AluOpType.max)
        else:
            h = CP // 2
            nc.vector.tensor_tensor(out_v[:, 0:h], pd_v[:, 0, 0:h],
                                    pd_v[:, 1, 0:h], op=mybir.AluOpType.max)
            split_out(0, h)
            nc.vector.tensor_tensor(out_v[:, h:CP], pd_v[:, 0, h:CP],
                                    pd_v[:, 1, h:CP], op=mybir.AluOpType.max)
            split_out(h, CP)
        return
    # Fused levels 1+2 of the 6-leaf tree as one 4-block in-place cascade:
    # block b computes max(slot[1+2b], slot[2b]) -> slot[7-b], i.e.
    #   (q2,q1)->7, (q4,q3)->6, (c,q5)->5, (out7,out6)->4
    # All three APs are affine (src stride 2*CP*L, out stride -CP*L) and
    # each block's reads complete a full CP*L-element stride before any
    # later block overwrites that region, so the overlap is stream-safe.
    sa = pd_v[:, 1:2].copy()
    sa.ap[1] = [2 * CP * L, 4]  # slots 1, 3, 5, 7
    sb = pd_v[:, 0:1].copy()
    sb.ap[1] = [2 * CP * L, 4]  # slots 0, 2, 4, 6
    so = pd_v[:, 7:8].copy()
    so.ap[1] = [-CP * L, 4]  # slots 7, 6, 5, 4
    nc.vector.tensor_tensor(so, sa, sb, op=mybir.AluOpType.max)
    if split_out is None:
        nc.vector.tensor_tensor(out_v, pd_v[:, 4], pd_v[:, 5],
                                op=mybir.AluOpType.max)
    else:
        # final-tile variant: emit the last max in two channel halves with
        # the store callback between them, so half the store overlaps the
        # second half of the compute (shrinks the pipeline tail).
        h = CP // 2
        nc.vector.tensor_tensor(out_v[:, 0:h], pd_v[:, 4, 0:h],
                                pd_v[:, 5, 0:h], op=mybir.AluOpType.max)
        split_out(0, h)
        nc.vector.tensor_tensor(out_v[:, h:CP], pd_v[:, 4, h:CP],
                                pd_v[:, 5, h:CP], op=mybir.AluOpType.max)
        split_out(h, CP)


def build_nc(bias_vals, reps=1):
    """Build the per-core Bass module. bias_vals: tuple of 5 floats (b_1..b_5).

    reps > 1 repeats the whole pipeline (same output) -- used only for
    differential timing experiments."""
    nc = bacc.Bacc("TRN2", target_bir_lowering=False, debug=False)
    imt = nc.dram_tensor("imt", [C, W, H], F16, kind="ExternalInput")
    biasneg = nc.dram_tensor("biasneg", [128, R], F32, kind="ExternalInput")
    iden = nc.dram_tensor("iden", [128, 128], F16, kind="ExternalInput")
    out = nc.dram_tensor("out", [C, H, W], F16, kind="ExternalOutput")

    with tile.TileContext(nc) as tc, ExitStack() as ctx:
        const_pool = ctx.enter_context(tc.tile_pool(name="const", bufs=1))
        hin_pool = ctx.enter_context(tc.tile_pool(name="hin", bufs=6))
        vin_pool = ctx.enter_context(tc.tile_pool(name="vin", bufs=5))
        pd_pool = ctx.enter_context(tc.tile_pool(name="pd", bufs=3))
        # pass 2 of group g is emitted after pass 1 of g+1 (software
        # pipelining), so two groups' worth of pass-1 results are live.
        hacc_pool = ctx.enter_context(tc.tile_pool(name="hacc", bufs=2 * nT + 1))
        st_pool = ctx.enter_context(tc.tile_pool(name="st", bufs=4))
        psf_pool = ctx.enter_context(tc.tile_pool(name="psf", bufs=3, space="PSUM"))

        identity = const_pool.tile([128, 128], F16)
        biasneg_t = const_pool.tile([128, R], F32)
        neg_t = const_pool.tile([128, CP * R], F16)
        nc.gpsimd.memset(neg_t[:], NEG)
        # identity/biasneg DMAs are deferred until the first image load is
        # in flight so they don't occupy HWDGE/DMA ahead of it (identity is
        # not needed until pass 2, biasneg not until the first subtracts).
        deferred_consts = [
            lambda: nc.sync.dma_start(identity[:], iden.ap()),
            lambda: nc.sync.dma_start(biasneg_t[:], biasneg.ap()),
        ]

        pad_seen = {}

        def set_pads(tile_, tag):
            """Write NEG into the halo pads; once per physical buffer if
            PADS_ONCE (pool buffers rotate round-robin, pads are never
            overwritten afterwards)."""
            n = pad_seen.get(tag, 0)
            if PADS_ONCE and n >= 6:
                return
            pad_seen[tag] = n + 1
            v = tile_[:].rearrange("p (s c) -> p s c", s=CP)
            src = neg_t[:].rearrange("p (s c) -> p s c", s=CP)
            nc.scalar.copy(v[:, :, 0:R], src)
            nc.scalar.copy(v[:, :, SL - R : SL], src)

        def alloc_pd(src3, dram_src=None, skip_center=False):
            """Allocate the 6-slot scratch and DMA the center tap into slot 5
            while other engines are busy. Pass 1 sources it straight from
            DRAM (independent of the halo load); pass 2 copies SBUF->SBUF
            from the transposed tile."""
            pd = pd_pool.tile([128, 8 * CP * L], F16, tag="pd")
            pd_v = pd[:].rearrange("p (d s c) -> p d s c", d=8, s=CP)
            if not skip_center:
                nc.sync.dma_start(
                    pd_v[:, R],
                    dram_src if dram_src is not None else src3[:, :, R : R + L],
                )
            return pd_v

        def dilate(eng, out_v, src3, pd_v, center_src=None, split_out=None):
            _dilate_dve(nc, pd_v, out_v, src3, biasneg_t, bias_vals,
                        center_src=center_src, split_out=split_out)

        def pass1(g):
            haccs = []
            for t in range(nT):
                ht = hin_pool.tile([128, CP * SL], F16, tag="hin")
                set_pads(ht, "hin")
                src = imt.ap()[
                    g * CP : (g + 1) * CP, t * 128 : (t + 1) * 128, :
                ].rearrange("c w h -> w c h")
                hv = ht[:].rearrange("p (s c) -> p s c", s=CP)
                nc.sync.dma_start(hv[:, :, R : R + L], src)
                while deferred_consts:
                    deferred_consts.pop()()
                first = g == 0 and t == 0
                pd_v = alloc_pd(hv, dram_src=src, skip_center=first)
                acc = hacc_pool.tile([128, CP * L], F16, tag="hacc")
                accv = acc[:].rearrange("p (s c) -> p s c", s=CP)
                dilate(_gp_tile(g, t), accv, hv, pd_v,
                       center_src=hv[:, :, R : R + L] if first else None)
                haccs.append(acc)
            return haccs

        def pass2(g, haccs):
            for t2 in range(nT):
                vt = vin_pool.tile([128, CP * SL], F16, tag="vin")
                set_pads(vt, "vin")
                vv = vt[:].rearrange("p (s c) -> p s c", s=CP)
                pt = psf_pool.tile([128, CP * L], F16, tag="psf")
                pt_v = pt[:].rearrange("p (s c) -> p s c", s=CP)
                for ci in range(CP):
                    for t1 in range(nT):
                        nc.tensor.transpose(
                            pt_v[:, ci, t1 * 128 : (t1 + 1) * 128],
                            haccs[t1][:, ci * L + t2 * 128 : ci * L + (t2 + 1) * 128],
                            identity[:],
                        )
                nc.scalar.copy(vv[:, :, R : R + L], pt_v)
                pd_v = alloc_pd(
                    vv, skip_center=(g == nG - 1 and t2 >= nT - 2)
                )
                stt = st_pool.tile([128, CP * L], F16, tag="st")
                stv = stt[:].rearrange("p (s c) -> p s c", s=CP)
                dst = out.ap()[
                    g * CP : (g + 1) * CP, t2 * 128 : (t2 + 1) * 128, :
                ].rearrange("c h w -> h c w")
                if g == nG - 1 and t2 == nT - 1:
                    def store_half(c0, c1):
                        nc.sync.dma_start(dst[:, c0:c1], stv[:, c0:c1])
                    dilate(_gp_tile(g, nT + t2), stv, vv, pd_v,
                           center_src=vv[:, :, R : R + L],
                           split_out=store_half)
                elif g == nG - 1 and t2 == nT - 2:
                    dilate(_gp_tile(g, nT + t2), stv, vv, pd_v,
                           center_src=vv[:, :, R : R + L])
                    nc.sync.dma_start(dst, stv)
                else:
                    dilate(_gp_tile(g, nT + t2), stv, vv, pd_v)
                    nc.sync.dma_start(dst, stv)

        for _rep in range(reps):
            # Software pipeline: pass 2 of group g is emitted after pass 1 of
            # group g+1, so every engine always has two groups of independent
            # work in its instruction window.
            prev = None
            for g in range(nG):
                haccs = pass1(g)
                if prev is not None:
                    pass2(prev[0], prev[1])
                prev = (g, haccs)
            pass2(prev[0], prev[1])

    nc.compile()
    return nc


_NC_CACHE = {}


def _get_nc(bias_vals=None):
    if bias_vals is None:
        bias_vals = next(iter(_NC_CACHE))
    if bias_vals not in _NC_CACHE:
        _NC_CACHE[bias_vals] = build_nc(bias_vals)
    return _NC_CACHE[bias_vals]


def _bias_vals(se_coef, se):
    se = np.asarray(se, dtype=np.float32)
    se_coef = np.asarray(se_coef, dtype=np.float32)
    bias11 = (se_coef * se[:, 0]).astype(np.float32)  # same fp32 op as reference
    return tuple(float(x) for x in bias11[R + 1 : 2 * R + 1])


def _make_in_maps(im, se_coef, se):
    im = np.asarray(im, dtype=np.float32)
    bias_vals = _bias_vals(se_coef, se)
    biasneg = np.ascontiguousarray(
        np.broadcast_to(-np.asarray(bias_vals, dtype=np.float32), (128, R))
    )
    iden = np.eye(128, dtype=np.float16)
    imt = np.ascontiguousarray(np.swapaxes(im, 2, 3)).astype(np.float16)
    return [
        {"imt": imt[b], "biasneg": biasneg, "iden": iden} for b in range(im.shape[0])
    ]


def kernel(im, se_coef, se):
    bias_vals = _bias_vals(se_coef, se)
    nc = _get_nc(bias_vals)
    in_maps = _make_in_maps(im, se_coef, se)
    res = run_bass_kernel_spmd(nc, in_maps, core_ids=list(range(N_CORES)))
    out = np.stack([res.results[b]["out"] for b in range(N_CORES)], axis=0)
    return out.astype(np.float32)
